# revision 5
# baseline (speedup 1.0000x reference)
"""FP8Linear kernel for Trainium2 (Bass/Tile), distributed over 8 NeuronCores.

Computation (matches the nn.Module reference):
    x:  [B=2, S=4096, K=4096] f32  -> x2d [M=8192, K]
    xq = tile_quant_dequant(x2d)    # per-row 1x64 chunks, fp8 e4m3fn round-trip
    wq = block_quant_dequant(w)     # 64x64 blocks of w [N=4096, K]
    out = f32(bf16(xq @ wq.T)) + bias  -> [B, S, N]

Distribution: 4x2 grid over (M, N). Each core independently computes a
[2048, 2048] output shard (data-parallel over rows, tensor-parallel over
out_features) -- no collectives.

v3 design (fully on-chip, m-tile-outer, wT resident):
  * natural-layout quantization: per 128-row tile, amax per 1x64 chunk
    (DVE reduce); for w, 64x64 block scales via PE-transpose of the
    chunk-amax + DVE reduce + a tiny DRAM broadcast bounce (all during
    the prologue, where the PE is idle anyway).
    s2 = max(amax,1e-12)/224; q = fp8e4(v*rs2); dq = bf16(q*s2), with
    the two multiplies alternating between DVE and GpSimd.
    (TRN fp8e4 max-normal is 240: v/(2s) with dequant by 2s reproduces
    the e4m3fn RNE grid for normals.)
  * dq tiles are transposed on-chip by the tensor engine (128x128 bf16
    transposes through PSUM, ACT evacuates groups of 4) -- no DRAM
    round-trip, no DMA-xbar transposes.
  * wT for the whole [4096 K, 2048 N] shard stays resident in SBUF
    (128 KiB/partition); xT tiles [128,32,128] stream through a
    4-deep pool, prepped two m-tiles ahead of consumption.
  * main loop: for each of 16 m-tiles, 4 panel sweeps of 32
    back-to-back N=512 matmuls (psum f32 accumulation, one psum bank
    live at a time), ACT evacuates f32 (skipping the reference's bf16
    round-trip of the output adds ~2e-3 relative error, well inside
    tolerance). Matmuls run dense, so the PE stays warm at 2.4 GHz.
  * bias is all-zeros in this problem (checked at run time; a general
    variant with a DVE bias add is built on demand).
"""

import sys

sys.path.insert(0, "/opt/trn_rl_repo")

import numpy as np
from contextlib import ExitStack

import concourse.bass as bass
import concourse.mybir as mybir
import concourse.tile as tile
from concourse import bacc
from concourse.bass import ts
from concourse.masks import make_identity

P = 128
QT = 64  # quantization chunk (1x64 for x, 64x64 blocks for w)

# full-problem dims
B, S, K, N = 2, 4096, 4096, 4096
M = B * S
# sharding grid
GRID_M, GRID_N = 4, 2
M_SH, N_SH = M // GRID_M, N // GRID_N  # 2048, 2048

F32 = mybir.dt.float32
BF16 = mybir.dt.bfloat16
FP8 = mybir.dt.float8e4

KQ = 1024          # processing quarter width (along K)
NQ = K // KQ       # 4 quarters per row-tile
CB = KQ // QT      # 16 scale chunks per quarter
KBF = K // QT      # 64 scale chunks per full row
KT = K // P        # 32 k-tiles of 128
N_PANEL = 512
N_PANELS = N_SH // N_PANEL   # 4
N_MT = M_SH // P             # 16 m-tiles
N_WT = N_SH // P             # 16 w row-tiles
X_AHEAD = 2                  # x tiles prepped ahead of their sweeps


class Ctx:
    """Bag of pools / constants shared by the emit helpers."""


def _quant_quarter(cx, nc, nat, dq, s2, rs2, c0, alt):
    """fp8 round-trip of one loaded quarter into dq (bf16), using scale
    columns [c0, c0+CB) of s2/rs2.  alt flips which of DVE/GpSimd does the
    quant vs dequant multiply so the two streams stay balanced."""
    nat_v = nat[:].rearrange("p (c t) -> p c t", t=QT)
    q = cx.q.tile([P, KQ], FP8, tag="q")
    q_v = q[:].rearrange("p (c t) -> p c t", t=QT)
    dq_v = dq[:].rearrange("p (c t) -> p c t", t=QT)
    e0, e1 = (nc.vector, nc.gpsimd) if alt else (nc.gpsimd, nc.vector)
    e0.tensor_tensor(
        q_v, nat_v,
        rs2[:, c0 : c0 + CB, None].to_broadcast((P, CB, QT)),
        op=mybir.AluOpType.mult,
    )
    e1.tensor_tensor(
        dq_v, q_v,
        s2[:, c0 : c0 + CB, None].to_broadcast((P, CB, QT)),
        op=mybir.AluOpType.mult,
    )


def _transpose_quarter(cx, nc, dq, dst, col0, qq):
    """PE-transpose one dq quarter [128, KQ] into dst[:, kt0:kt0+8,
    col0:col0+128] (a [128, 32, ncol] K-major cache tile)."""
    kt0 = qq * (KQ // P)
    for g in range(KQ // P // 4):  # groups of 4 k-subtiles per PSUM tile
        tp = cx.tp.tile([P, 4 * P], BF16, tag="tp")
        for i in range(4):
            nc.tensor.transpose(
                tp[:, i * P : (i + 1) * P],
                dq[:, (g * 4 + i) * P : (g * 4 + i + 1) * P],
                cx.ident_bf16[:],
            )
        nc.scalar.copy(
            dst[:, kt0 + g * 4 : kt0 + g * 4 + 4, col0 : col0 + P],
            tp[:].rearrange("p (i m) -> p i m", m=P),
        )


def _emit_x_tile(cx, nc, x, mt):
    """Quantize + transpose x rows [mt*128, +128) into a streamed xT tile."""
    xT = cx.xT.tile([P, KT, P], BF16, tag="xT", name=f"xT{mt}")
    cx.xTs[mt] = xT
    row0 = mt * P
    for qq in range(NQ):
        nat = cx.nat.tile([P, KQ], F32, tag="nat")
        nc.sync.dma_start(nat[:], x[row0 : row0 + P, qq * KQ : (qq + 1) * KQ])
        a = cx.amax.tile([P, CB], F32, tag="amax_x")
        nc.vector.tensor_reduce(
            a[:], nat[:].rearrange("p (c t) -> p c t", t=QT),
            axis=mybir.AxisListType.X, op=mybir.AluOpType.max,
            apply_absolute_value=True,
        )
        s2 = cx.scale.tile([P, CB], F32, tag="s2x")
        rs2 = cx.scale.tile([P, CB], F32, tag="rs2x")
        nc.vector.tensor_scalar(
            s2[:], a[:], 1e-12, 1.0 / 224.0,
            op0=mybir.AluOpType.max, op1=mybir.AluOpType.mult,
        )
        nc.vector.reciprocal(rs2[:], s2[:])
        dq = cx.dq.tile([P, KQ], BF16, tag="dq")
        _quant_quarter(cx, nc, nat, dq, s2, rs2, 0, alt=(qq % 2 == 0))
        _transpose_quarter(cx, nc, dq, xT, 0, qq)


def _emit_w_tile(cx, nc, w, rt):
    """Quantize + transpose w rows [rt*128, +128) into resident
    wT[:, :, rt*128:+128] with 64x64 block scales."""
    row0 = rt * P
    nats = []
    a = cx.amax.tile([P, KBF], F32, tag="amax_w")
    for qq in range(NQ):
        nat = cx.nat.tile([P, KQ], F32, tag="nat")
        nats.append(nat)
        nc.sync.dma_start(nat[:], w[row0 : row0 + P, qq * KQ : (qq + 1) * KQ])
        nc.vector.tensor_reduce(
            a[:, qq * CB : (qq + 1) * CB],
            nat[:].rearrange("p (c t) -> p c t", t=QT),
            axis=mybir.AxisListType.X, op=mybir.AluOpType.max,
            apply_absolute_value=True,
        )
    # 64-partition-group max via PE transpose (f32) + DVE reduce; the
    # [KBF, 2] block scales bounce through DRAM to become
    # partition-broadcast rows.
    at_ps = cx.tps.tile([KBF, P], F32, tag="at_ps")
    nc.tensor.transpose(at_ps[:], a[:], cx.ident_f32[:])
    r = cx.amax.tile([KBF, 2], F32, tag="r_blk")
    nc.vector.tensor_reduce(
        r[:], at_ps[:].rearrange("p (g t) -> p g t", t=QT),
        axis=mybir.AxisListType.X, op=mybir.AluOpType.max,
    )
    s2blk = cx.scale.tile([KBF, 2], F32, tag="s2blk")
    nc.vector.tensor_scalar(
        s2blk[:], r[:], 1e-12, 1.0 / 224.0,
        op0=mybir.AluOpType.max, op1=mybir.AluOpType.mult,
    )
    s2 = cx.scale.tile([P, KBF], F32, tag="s2w")
    rs2 = cx.scale.tile([P, KBF], F32, tag="rs2w")
    for nb in (0, 1):
        s2row_dram = cx.dram_small.tile([1, KBF], F32, tag="s2row_dram")
        nc.gpsimd.dma_start(s2row_dram[:], s2blk[:, nb : nb + 1])
        nc.gpsimd.dma_start(
            s2[nb * QT : (nb + 1) * QT, :],
            s2row_dram[:].to_broadcast((QT, KBF)),
        )
    nc.vector.reciprocal(rs2[:], s2[:])
    for qq in range(NQ):
        dq = cx.dq.tile([P, KQ], BF16, tag="dq")
        _quant_quarter(cx, nc, nats[qq], dq, s2, rs2, qq * CB,
                       alt=(qq % 2 == 0))
        _transpose_quarter(cx, nc, dq, cx.wT, rt * P, qq)


def _emit_sweep(cx, nc, out, mt):
    """All matmuls for one m-tile: 4 panels x 32 k-tiles, one psum bank
    live at a time, ACT evacuation + store."""
    xT = cx.xTs[mt]
    for pn in range(N_PANELS):
        ps = cx.mm.tile([P, N_PANEL], F32, tag="mm")
        for kt in range(KT):
            nc.tensor.matmul(
                ps[:], xT[:, kt, :], cx.wT[:, kt, ts(pn, N_PANEL)],
                start=(kt == 0), stop=(kt == KT - 1),
            )
        outf = cx.outf.tile([P, N_PANEL], F32, tag="outf")
        if cx.with_bias:
            nc.vector.tensor_tensor(
                outf[:], ps[:], cx.bias_bc[:, ts(pn, N_PANEL)],
                op=mybir.AluOpType.add,
            )
        else:
            nc.scalar.copy(outf[:], ps[:])
        nc.scalar.dma_start(
            out[mt * P : (mt + 1) * P, ts(pn, N_PANEL)], outf[:]
        )


def fp8_linear_core_kernel(tc, out, x, w, b, with_bias):
    nc = tc.nc
    ctx = tc.ctx  # ExitStack owned by the caller

    cx = Ctx()
    cx.with_bias = with_bias
    cx.xTs = [None] * N_MT

    cx.nat = ctx.enter_context(tc.tile_pool(name="nat", bufs=5))
    cx.q = ctx.enter_context(tc.tile_pool(name="q", bufs=3))
    cx.dq = ctx.enter_context(tc.tile_pool(name="dq", bufs=4))
    cx.amax = ctx.enter_context(tc.tile_pool(name="amax", bufs=6))
    cx.scale = ctx.enter_context(tc.tile_pool(name="scale", bufs=8))
    cx.xT = ctx.enter_context(tc.tile_pool(name="xT", bufs=4))
    cx.wTp = ctx.enter_context(tc.tile_pool(name="wTp", bufs=1))
    cx.outf = ctx.enter_context(tc.tile_pool(name="outf", bufs=3))
    cx.const = ctx.enter_context(tc.tile_pool(name="const", bufs=1))
    cx.mm = ctx.enter_context(tc.tile_pool(name="mm", bufs=4, space="PSUM"))
    cx.tp = ctx.enter_context(tc.tile_pool(name="tp", bufs=2, space="PSUM"))
    cx.tps = ctx.enter_context(tc.tile_pool(name="tps", bufs=1, space="PSUM"))
    cx.dram_small = ctx.enter_context(
        tc.tile_pool(name="scratch_s", bufs=8, space="DRAM")
    )

    cx.wT = cx.wTp.tile([P, KT, N_SH], BF16, name="wT")

    cx.ident_f32 = cx.const.tile([P, P], F32)
    make_identity(nc, cx.ident_f32)
    cx.ident_bf16 = cx.const.tile([P, P], BF16)
    make_identity(nc, cx.ident_bf16)

    if with_bias:
        cx.biasp = ctx.enter_context(tc.tile_pool(name="biasp", bufs=1))
        cx.bias_bc = cx.biasp.tile([P, N_SH], F32)
        nc.sync.dma_start(cx.bias_bc[:], b[:].to_broadcast((P, N_SH)))

    # Prologue: the whole weight shard is quantized + transposed into the
    # resident wT (the PE only has the transposes to do here, so the
    # quant engines pace this phase), plus the first x tiles.
    for rt in range(N_WT):
        _emit_w_tile(cx, nc, w, rt)
    for mt in range(X_AHEAD):
        _emit_x_tile(cx, nc, x, mt)

    # Main: one m-tile at a time; its xT streams in two tiles ahead.
    for mt in range(N_MT):
        if mt + X_AHEAD < N_MT:
            _emit_x_tile(cx, nc, x, mt + X_AHEAD)
        _emit_sweep(cx, nc, out, mt)


def build_core_bass(with_bias=False, num_devices=8):
    """Build the (SPMD-identical) per-core Bass program."""
    nc = bacc.Bacc(
        "TRN2", target_bir_lowering=False, debug=False, num_devices=num_devices
    )
    x = nc.dram_tensor("x", [M_SH, K], F32, kind="ExternalInput").ap()
    w = nc.dram_tensor("w", [N_SH, K], F32, kind="ExternalInput").ap()
    b = nc.dram_tensor("b", [1, N_SH], F32, kind="ExternalInput").ap()
    out = nc.dram_tensor("out", [M_SH, N_SH], F32, kind="ExternalOutput").ap()
    with tile.TileContext(nc) as tc:
        with ExitStack() as stack:
            tc.ctx = stack
            fp8_linear_core_kernel(tc, out, x, w, b, with_bias)
    nc.compile()
    return nc


_NC_CACHE = {}


def _get_nc(with_bias):
    if with_bias not in _NC_CACHE:
        _NC_CACHE[with_bias] = build_core_bass(with_bias=with_bias)
    return _NC_CACHE[with_bias]


def kernel(x, weight, bias):
    """Full-problem entry point: x [2,4096,4096] f32, weight [4096,4096] f32,
    bias [4096] f32 -> [2,4096,4096] f32."""
    from concourse.bass_utils import run_bass_kernel_spmd

    x2d = np.ascontiguousarray(x.reshape(M, K), dtype=np.float32)
    weight = np.ascontiguousarray(weight, dtype=np.float32)
    bias2d = np.ascontiguousarray(bias.reshape(1, N), dtype=np.float32)

    with_bias = bool(np.any(bias2d))
    nc = _get_nc(with_bias)

    in_maps = []
    for core in range(8):
        mi, nj = core // GRID_N, core % GRID_N
        in_maps.append(
            {
                "x": np.ascontiguousarray(x2d[mi * M_SH : (mi + 1) * M_SH]),
                "w": np.ascontiguousarray(weight[nj * N_SH : (nj + 1) * N_SH]),
                "b": np.ascontiguousarray(bias2d[:, nj * N_SH : (nj + 1) * N_SH]),
            }
        )

    res = run_bass_kernel_spmd(nc, in_maps, core_ids=list(range(8)))
    global LAST_EXEC_TIME_NS
    LAST_EXEC_TIME_NS = res.exec_time_ns

    out = np.empty((M, N), dtype=np.float32)
    for core in range(8):
        mi, nj = core // GRID_N, core % GRID_N
        out[mi * M_SH : (mi + 1) * M_SH, nj * N_SH : (nj + 1) * N_SH] = (
            res.results[core]["out"]
        )
    return out.reshape(B, S, N)


# revision 8
# speedup vs baseline: 1.1138x; 1.1138x over previous
"""FP8Linear kernel for Trainium2 (Bass/Tile), distributed over 8 NeuronCores.

Computation (matches the nn.Module reference):
    x:  [B=2, S=4096, K=4096] f32  -> x2d [M=8192, K]
    xq = tile_quant_dequant(x2d)    # per-row 1x64 chunks, fp8 e4m3fn round-trip
    wq = block_quant_dequant(w)     # 64x64 blocks of w [N=4096, K]
    out = f32(bf16(xq @ wq.T)) + bias  -> [B, S, N]

Distribution: 4x2 grid over (M, N). Each core independently computes a
[2048, 2048] output shard (data-parallel over rows, tensor-parallel over
out_features) -- no collectives.

v3 design (fully on-chip, m-tile-outer, wT resident):
  * natural-layout quantization: per 128-row tile, amax per 1x64 chunk
    (DVE reduce); for w, 64x64 block scales via PE-transpose of the
    chunk-amax + DVE reduce + a tiny DRAM broadcast bounce (all during
    the prologue, where the PE is idle anyway).
    s2 = max(amax,1e-12)/224; q = fp8e4(v*rs2); dq = bf16(q*s2), with
    the two multiplies alternating between DVE and GpSimd.
    (TRN fp8e4 max-normal is 240: v/(2s) with dequant by 2s reproduces
    the e4m3fn RNE grid for normals.)
  * dq tiles are transposed on-chip by the tensor engine (128x128 bf16
    transposes through PSUM, ACT evacuates groups of 4) -- no DRAM
    round-trip, no DMA-xbar transposes.
  * wT for the whole [4096 K, 2048 N] shard stays resident in SBUF
    (128 KiB/partition); xT tiles [128,32,128] stream through a
    4-deep pool, prepped two m-tiles ahead of consumption.
  * main loop: for each of 16 m-tiles, 4 panel sweeps of 32
    back-to-back N=512 matmuls (psum f32 accumulation, one psum bank
    live at a time), ACT evacuates f32 (skipping the reference's bf16
    round-trip of the output adds ~2e-3 relative error, well inside
    tolerance). Matmuls run dense, so the PE stays warm at 2.4 GHz.
  * bias is all-zeros in this problem (checked at run time; a general
    variant with a DVE bias add is built on demand).
"""

import sys

sys.path.insert(0, "/opt/trn_rl_repo")

import numpy as np
from contextlib import ExitStack

import concourse.bass as bass
import concourse.mybir as mybir
import concourse.tile as tile
from concourse import bacc
from concourse.bass import ts
from concourse.masks import make_identity

P = 128
QT = 64  # quantization chunk (1x64 for x, 64x64 blocks for w)

# full-problem dims
B, S, K, N = 2, 4096, 4096, 4096
M = B * S
# sharding grid
GRID_M, GRID_N = 4, 2
M_SH, N_SH = M // GRID_M, N // GRID_N  # 2048, 2048

F32 = mybir.dt.float32
BF16 = mybir.dt.bfloat16
FP8 = mybir.dt.float8e4

KQ = 1024          # processing quarter width (along K)
NQ = K // KQ       # 4 quarters per row-tile
CB = KQ // QT      # 16 scale chunks per quarter
KBF = K // QT      # 64 scale chunks per full row
KT = K // P        # 32 k-tiles of 128
N_PANEL = 512
N_PANELS = N_SH // N_PANEL   # 4
N_MT = M_SH // P             # 16 m-tiles
N_WT = N_SH // P             # 16 w row-tiles
X_AHEAD = 2                  # x tiles prepped ahead of their sweeps


class Ctx:
    """Bag of pools / constants shared by the emit helpers."""


def _quant_quarter(cx, nc, nat, dq, s2, rs2, c0, eng):
    """fp8 round-trip of one loaded quarter into dq (bf16), using scale
    columns [c0, c0+CB) of s2/rs2.  Both multiplies run on `eng` so there
    is no cross-engine ping-pong inside the chain (in-order queues)."""
    nat_v = nat[:].rearrange("p (c t) -> p c t", t=QT)
    q = cx.q.tile([P, KQ], FP8, tag="q")
    q_v = q[:].rearrange("p (c t) -> p c t", t=QT)
    dq_v = dq[:].rearrange("p (c t) -> p c t", t=QT)
    eng.tensor_tensor(
        q_v, nat_v,
        rs2[:, c0 : c0 + CB, None].to_broadcast((P, CB, QT)),
        op=mybir.AluOpType.mult,
    )
    eng.tensor_tensor(
        dq_v, q_v,
        s2[:, c0 : c0 + CB, None].to_broadcast((P, CB, QT)),
        op=mybir.AluOpType.mult,
    )


def _transpose_quarter(cx, nc, dq, dst, col0, qq):
    """PE-transpose one dq quarter [128, KQ] into dst[:, kt0:kt0+8,
    col0:col0+128] (a [128, 32, ncol] K-major cache tile)."""
    kt0 = qq * (KQ // P)
    for g in range(KQ // P // 4):  # groups of 4 k-subtiles per PSUM tile
        tp = cx.tp.tile([P, 4 * P], BF16, tag="tp")
        for i in range(4):
            nc.tensor.transpose(
                tp[:, i * P : (i + 1) * P],
                dq[:, (g * 4 + i) * P : (g * 4 + i + 1) * P],
                cx.ident_bf16[:],
            )
        nc.scalar.copy(
            dst[:, kt0 + g * 4 : kt0 + g * 4 + 4, col0 : col0 + P],
            tp[:].rearrange("p (i m) -> p i m", m=P),
        )


def _emit_x_tile(cx, nc, x, mt):
    """Quantize + transpose x rows [mt*128, +128) into a streamed xT tile."""
    xT = cx.xT.tile([P, KT, P], BF16, tag="xT", name=f"xT{mt}")
    cx.xTs[mt] = xT
    row0 = mt * P
    nats = []
    a = cx.amax.tile([P, KBF], F32, tag="amax_x")
    for qq in range(NQ):
        nat = cx.nat.tile([P, KQ], F32, tag="nat")
        nats.append(nat)
        nc.sync.dma_start(nat[:], x[row0 : row0 + P, qq * KQ : (qq + 1) * KQ])
        nc.vector.tensor_reduce(
            a[:, qq * CB : (qq + 1) * CB],
            nat[:].rearrange("p (c t) -> p c t", t=QT),
            axis=mybir.AxisListType.X, op=mybir.AluOpType.max,
            apply_absolute_value=True,
        )
    s2 = cx.scale.tile([P, KBF], F32, tag="s2x")
    rs2 = cx.scale.tile([P, KBF], F32, tag="rs2x")
    nc.vector.tensor_scalar(
        s2[:], a[:], 1e-12, 1.0 / 224.0,
        op0=mybir.AluOpType.max, op1=mybir.AluOpType.mult,
    )
    nc.vector.reciprocal(rs2[:], s2[:])
    for qq in range(NQ):
        dq = cx.dq.tile([P, KQ], BF16, tag="dq")
        eng = nc.vector if qq < 2 else nc.gpsimd
        _quant_quarter(cx, nc, nats[qq], dq, s2, rs2, qq * CB, eng)
        _transpose_quarter(cx, nc, dq, xT, 0, qq)


def _emit_w_head(cx, nc, w, rt):
    """Loads + chunk amax + 64x64 block scales for w rows [rt*128, +128).
    Returns state for _emit_w_tail.  The scale broadcast bounces through
    DRAM (SWDGE); recip is deferred to the tail so the bounce latency is
    hidden behind the previous tile's quant work."""
    row0 = rt * P
    nats = []
    a = cx.amax.tile([P, KBF], F32, tag="amax_w")
    for qq in range(NQ):
        nat = cx.nat.tile([P, KQ], F32, tag="nat")
        nats.append(nat)
        nc.sync.dma_start(nat[:], w[row0 : row0 + P, qq * KQ : (qq + 1) * KQ])
        nc.vector.tensor_reduce(
            a[:, qq * CB : (qq + 1) * CB],
            nat[:].rearrange("p (c t) -> p c t", t=QT),
            axis=mybir.AxisListType.X, op=mybir.AluOpType.max,
            apply_absolute_value=True,
        )
    at_ps = cx.tps.tile([KBF, P], F32, tag="at_ps")
    nc.tensor.transpose(at_ps[:], a[:], cx.ident_f32[:])
    r = cx.amax.tile([KBF, 2], F32, tag="r_blk")
    nc.vector.tensor_reduce(
        r[:], at_ps[:].rearrange("p (g t) -> p g t", t=QT),
        axis=mybir.AxisListType.X, op=mybir.AluOpType.max,
    )
    s2blk = cx.scale.tile([KBF, 2], F32, tag="s2blk")
    nc.vector.tensor_scalar(
        s2blk[:], r[:], 1e-12, 1.0 / 224.0,
        op0=mybir.AluOpType.max, op1=mybir.AluOpType.mult,
    )
    s2 = cx.scale.tile([P, KBF], F32, tag="s2w")
    rs2 = cx.scale.tile([P, KBF], F32, tag="rs2w")
    for nb in (0, 1):
        s2row_dram = cx.dram_small.tile([1, KBF], F32, tag="s2row_dram")
        nc.gpsimd.dma_start(s2row_dram[:], s2blk[:, nb : nb + 1])
        nc.gpsimd.dma_start(
            s2[nb * QT : (nb + 1) * QT, :],
            s2row_dram[:].to_broadcast((QT, KBF)),
        )
    return (rt, nats, s2, rs2)


def _emit_w_tail(cx, nc, state):
    rt, nats, s2, rs2 = state
    nc.vector.reciprocal(rs2[:], s2[:])
    for qq in range(NQ):
        dq = cx.dq.tile([P, KQ], BF16, tag="dq")
        eng = nc.vector if qq < 2 else nc.gpsimd
        _quant_quarter(cx, nc, nats[qq], dq, s2, rs2, qq * CB, eng)
        _transpose_quarter(cx, nc, dq, cx.wT, rt * P, qq)


def _emit_sweep(cx, nc, out, mt):
    """All matmuls for one m-tile: 4 panels x 32 k-tiles, one psum bank
    live at a time, ACT evacuation + store."""
    xT = cx.xTs[mt]
    for pn in range(N_PANELS):
        ps = cx.mm.tile([P, N_PANEL], F32, tag="mm")
        for kt in range(KT):
            nc.tensor.matmul(
                ps[:], xT[:, kt, :], cx.wT[:, kt, ts(pn, N_PANEL)],
                start=(kt == 0), stop=(kt == KT - 1),
            )
        outf = cx.outf.tile([P, N_PANEL], F32, tag="outf")
        if cx.with_bias:
            nc.vector.tensor_tensor(
                outf[:], ps[:], cx.bias_bc[:, ts(pn, N_PANEL)],
                op=mybir.AluOpType.add,
            )
        else:
            nc.scalar.copy(outf[:], ps[:])
        nc.scalar.dma_start(
            out[mt * P : (mt + 1) * P, ts(pn, N_PANEL)], outf[:]
        )


def fp8_linear_core_kernel(tc, out, x, w, b, with_bias):
    nc = tc.nc
    ctx = tc.ctx  # ExitStack owned by the caller

    cx = Ctx()
    cx.with_bias = with_bias
    cx.xTs = [None] * N_MT

    cx.nat = ctx.enter_context(tc.tile_pool(name="nat", bufs=5))
    cx.q = ctx.enter_context(tc.tile_pool(name="q", bufs=2))
    cx.dq = ctx.enter_context(tc.tile_pool(name="dq", bufs=3))
    cx.amax = ctx.enter_context(tc.tile_pool(name="amax", bufs=6))
    cx.scale = ctx.enter_context(tc.tile_pool(name="scale", bufs=8))
    cx.xT = ctx.enter_context(tc.tile_pool(name="xT", bufs=4))
    cx.wTp = ctx.enter_context(tc.tile_pool(name="wTp", bufs=1))
    cx.outf = ctx.enter_context(tc.tile_pool(name="outf", bufs=2))
    cx.const = ctx.enter_context(tc.tile_pool(name="const", bufs=1))
    cx.mm = ctx.enter_context(tc.tile_pool(name="mm", bufs=4, space="PSUM"))
    cx.tp = ctx.enter_context(tc.tile_pool(name="tp", bufs=2, space="PSUM"))
    cx.tps = ctx.enter_context(tc.tile_pool(name="tps", bufs=2, space="PSUM"))
    cx.dram_small = ctx.enter_context(
        tc.tile_pool(name="scratch_s", bufs=8, space="DRAM")
    )

    cx.wT = cx.wTp.tile([P, KT, N_SH], BF16, name="wT")

    cx.ident_f32 = cx.const.tile([P, P], F32)
    make_identity(nc, cx.ident_f32)
    cx.ident_bf16 = cx.const.tile([P, P], BF16)
    make_identity(nc, cx.ident_bf16)

    if with_bias:
        cx.biasp = ctx.enter_context(tc.tile_pool(name="biasp", bufs=1))
        cx.bias_bc = cx.biasp.tile([P, N_SH], F32)
        nc.sync.dma_start(cx.bias_bc[:], b[:].to_broadcast((P, N_SH)))

    # Prologue: the whole weight shard is quantized + transposed into the
    # resident wT (the PE only has the transposes to do here, so the
    # quant engines pace this phase), plus the first x tiles.
    state = _emit_w_head(cx, nc, w, 0)
    for rt in range(1, N_WT):
        nxt = _emit_w_head(cx, nc, w, rt)
        _emit_w_tail(cx, nc, state)
        state = nxt
    _emit_w_tail(cx, nc, state)
    for mt in range(X_AHEAD):
        _emit_x_tile(cx, nc, x, mt)

    # Main: one m-tile at a time; its xT streams in two tiles ahead.
    for mt in range(N_MT):
        if mt + X_AHEAD < N_MT:
            _emit_x_tile(cx, nc, x, mt + X_AHEAD)
        _emit_sweep(cx, nc, out, mt)


def build_core_bass(with_bias=False, num_devices=8):
    """Build the (SPMD-identical) per-core Bass program."""
    nc = bacc.Bacc(
        "TRN2", target_bir_lowering=False, debug=False, num_devices=num_devices
    )
    x = nc.dram_tensor("x", [M_SH, K], F32, kind="ExternalInput").ap()
    w = nc.dram_tensor("w", [N_SH, K], F32, kind="ExternalInput").ap()
    b = nc.dram_tensor("b", [1, N_SH], F32, kind="ExternalInput").ap()
    out = nc.dram_tensor("out", [M_SH, N_SH], F32, kind="ExternalOutput").ap()
    with tile.TileContext(nc) as tc:
        with ExitStack() as stack:
            tc.ctx = stack
            fp8_linear_core_kernel(tc, out, x, w, b, with_bias)
    nc.compile()
    return nc


_NC_CACHE = {}


def _get_nc(with_bias):
    if with_bias not in _NC_CACHE:
        _NC_CACHE[with_bias] = build_core_bass(with_bias=with_bias)
    return _NC_CACHE[with_bias]


def kernel(x, weight, bias):
    """Full-problem entry point: x [2,4096,4096] f32, weight [4096,4096] f32,
    bias [4096] f32 -> [2,4096,4096] f32."""
    from concourse.bass_utils import run_bass_kernel_spmd

    x2d = np.ascontiguousarray(x.reshape(M, K), dtype=np.float32)
    weight = np.ascontiguousarray(weight, dtype=np.float32)
    bias2d = np.ascontiguousarray(bias.reshape(1, N), dtype=np.float32)

    with_bias = bool(np.any(bias2d))
    nc = _get_nc(with_bias)

    in_maps = []
    for core in range(8):
        mi, nj = core // GRID_N, core % GRID_N
        in_maps.append(
            {
                "x": np.ascontiguousarray(x2d[mi * M_SH : (mi + 1) * M_SH]),
                "w": np.ascontiguousarray(weight[nj * N_SH : (nj + 1) * N_SH]),
                "b": np.ascontiguousarray(bias2d[:, nj * N_SH : (nj + 1) * N_SH]),
            }
        )

    res = run_bass_kernel_spmd(nc, in_maps, core_ids=list(range(8)))
    global LAST_EXEC_TIME_NS
    LAST_EXEC_TIME_NS = res.exec_time_ns

    out = np.empty((M, N), dtype=np.float32)
    for core in range(8):
        mi, nj = core // GRID_N, core % GRID_N
        out[mi * M_SH : (mi + 1) * M_SH, nj * N_SH : (nj + 1) * N_SH] = (
            res.results[core]["out"]
        )
    return out.reshape(B, S, N)
